# revision 26
# baseline (speedup 1.0000x reference)
"""Row-wise argmax + label lookup kernel for Trainium2 (8 NeuronCores).

Problem: inputs (16777216, 8) f32, label_table (8,) int32.
    y[i] = label_table[argmax(inputs[i, :])]   (first-occurrence ties)

Sharding: rows split evenly across 8 cores (data parallel, no comms).
Each core streams its 64 MiB slice through SBUF in 4 MiB tiles, and a
SINGLE custom Vector-engine instruction per tile computes the
first-occurrence argmax of every 8-element row in one streaming pass:

    rm   = scan(MAX, Src1)           Src1 = x shifted one element back;
                                     a hand-built FSM step-state resets rm
                                     to -FLT_MAX at every 8-element page
                                     boundary -> exact exclusive per-row
                                     running max (raw f32 compares)
    q    = Src0 > rm                 strict increase of the row prefix max;
                                     the LAST strict increase in a row is
                                     the first occurrence of the row max
    cand = q * (scan(ADD, 1) + 1)    candidate scores = global position + 2,
                                     ascending, so the global running max
    run  = scan(MAX, cand, init=1)   self-segments across rows

run is written COMPACT (out access pattern repeats each row's address 8x;
the last write per row wins): run[row] = 8*row_in_tile + argmax + 2.
The decode and the tiny label lookup happen on the host. The first row of
each (partition, tile) block reads one uninitialized pad element in the
shifted stream; those 16K rows are recomputed exactly on the host.
"""

import numpy as np

N_CORES = 8
ROWS = 16777216
C = 8
ROWS_PER_CORE = ROWS // N_CORES  # 2_097_152
P = 128
TILE_F = 8192  # f32 elements per partition per tile (32 KiB)
GROUPS = TILE_F // C  # 1024 rows per partition per tile
ROWS_PER_TILE = P * GROUPS  # 131_072
N_TILES = ROWS_PER_CORE // ROWS_PER_TILE  # 16
PAD = 8  # pad elements before the data; keeps the DMA destination aligned

_NC_CACHE = {}
_REGISTERED = {}

FLT_MAX = float(np.finfo(np.float32).max)


def _group_argmax_ref(in0, in1, s0, s1, imm2):
    x = np.asarray(in0, np.float32)
    Pp, S, N = x.shape
    xs = np.asarray(in1, np.float32).reshape(x.shape)
    rm = np.empty_like(x)
    rm[:, 0, 0] = xs[:, 0, 0]          # page 0 k=0: steady, max(-FLT_MAX, pad)
    rm[:, 1:, 0] = -FLT_MAX            # later pages: step-state reset
    for k in range(1, N):
        rm[:, :, k] = np.maximum(rm[:, :, k - 1], xs[:, :, k])
    q = (x > rm).astype(np.float32)
    gidx = (np.arange(S * N, dtype=np.float32) + 2.0).reshape(1, S, N)
    cand = q * gidx
    run = np.maximum.accumulate(cand.reshape(Pp, S * N), axis=1).reshape(x.shape)
    return np.maximum(run, 1.0).astype(np.float32)  # run scan init = One


def _get_group_argmax_op():
    """Build + register the one-pass grouped-argmax custom DVE op.

    The framework's lower() cannot express a resetting MAX scan (subdim
    scans hold in steady state), so the 3-state FSM (seed/steady/step) is
    assembled manually and the DveOp compile cache is pre-seeded.
    """
    if "op" in _REGISTERED:
        return _REGISTERED["op"]

    from concourse import dve_ops, dve_spec as ds
    from concourse.dve_ops import DveOp, _COMPILE_CACHE
    from concourse.dve_spec import AluOp, MaxNeg, One, Scan, Spec, Src0, Src1
    from concourse.dve_uop import DveOpSpec

    name = "GROUP_ARGMAX1P_ANT"

    rm = Scan(AluOp.MAX, Src1)
    q = Src0 > rm
    idxp = Scan(AluOp.ADD, One, init=One)
    cand = q * idxp
    run = Scan(AluOp.MAX, One, init=One)
    object.__setattr__(run, "expr", cand)   # bypass conservative nesting check
    spec = Spec(body=run, reference=_group_argmax_ref)

    opcode = dve_ops._CUSTOM_DVE_ROW_BASE + len(dve_ops.OPS)
    assert opcode < 0x20
    dve_ops._SUB_OPCODE_FOR_NAME[name] = opcode

    shas = {}
    for ver in ("v3", "v4"):
        spec2 = ds._hoist_stream_invariant_ops(spec)
        scans = ds._collect(spec2.body, Scan)
        p = ds._build_placement(spec2, scans, ds.N_STAGES[ver], ds.N_LANES[ver])
        seed_ov, step_ov = ds._scan_overrides(scans, p.node_stage)
        assert not step_ov
        rm2 = [s for s in scans if s.op == AluOp.MAX and s.expr is Src1]
        assert len(rm2) == 1, scans
        my_step_ov = {p.node_stage[rm2[0]]: ds._Stage(AluOp.BYPASS, MaxNeg)}

        body_lvs = ds._body_scan_leaves(spec2)
        consume = (Src0 in body_lvs, Src1 in body_lvs)
        assert consume == (True, True)

        T = ds.Trigger
        states = [
            ds._State(placement=p, overrides=seed_ov, trigger=ds.COUNT_ONCE,
                      repeat=1, next=(1, 0, 0), write_out=False),
            ds._State(placement=p, consume=consume,
                      trigger=(T.SRC_TENSOR_DONE, T.SUB_DIM_DONE, T.NONE),
                      next=(0, 2, 0)),
            ds._State(placement=p, consume=consume, overrides=my_step_ov,
                      trigger=(T.SRC_TENSOR_DONE, T.SUB_DIM_DONE, T.COUNT),
                      next=(0, 2, 1), repeat=1),
        ]
        uops = [ds._assemble(s) for s in states]
        for u in uops:
            u.validate(ver)
        compiled = DveOpSpec(name=name, uops=uops, opcode=opcode, rd1_en=True)
        shas[ver] = compiled.sha(ver)
        _COMPILE_CACHE[(name, ver)] = compiled

    op = DveOp(name, spec, subdim=True, uops_sha=shas)
    dve_ops.OPS.append(op)
    dve_ops.CUSTOM_DVE_SPECS[name] = spec
    _REGISTERED["op"] = op
    return op


def _tile_plan(n_tiles):
    """List of (elems_per_partition, groups_per_partition) per device tile.

    The first full tile's worth of data is split into progressively larger
    chunks so the Vector engine starts within a few microseconds of launch.
    """
    plan = [(TILE_F // 16, GROUPS // 16)] * 4   # 4 x 256 KiB
    plan += [(TILE_F // 4, GROUPS // 4)] * 3    # 3 x 1 MiB
    plan += [(TILE_F, GROUPS)] * (n_tiles - 1)
    assert sum(tf for tf, _ in plan) == n_tiles * TILE_F
    return plan


def _build_nc(n_tiles=N_TILES):
    import concourse.tile as tile
    from concourse import bacc, mybir

    f32 = mybir.dt.float32
    u16 = mybir.dt.uint16
    u8 = mybir.dt.uint8
    Alu = mybir.AluOpType
    argmax_op = _get_group_argmax_op()

    rows = n_tiles * ROWS_PER_TILE
    nc = bacc.Bacc("TRN2", target_bir_lowering=False)
    x = nc.dram_tensor("x", [rows * C], f32, kind="ExternalInput")
    y = nc.dram_tensor("y", [rows], u8, kind="ExternalOutput")

    plan = _tile_plan(n_tiles)
    BUFS = 4

    with tile.TileContext(nc) as tc:
        with tc.tile_pool(name="xp", bufs=BUFS) as xp, \
             tc.tile_pool(name="rp", bufs=BUFS) as rp, \
             tc.tile_pool(name="op_", bufs=BUFS) as op_, \
             tc.tile_pool(name="cst", bufs=1) as cst:
            # i82[p, j] = 8j + 2 (score offsets for the on-device decode)
            i82 = cst.tile([P, GROUPS], u16)
            nc.gpsimd.iota(i82[:, :], [[C, GROUPS]], channel_multiplier=0,
                           allow_small_or_imprecise_dtypes=True)
            nc.vector.tensor_scalar_add(i82[:, :], i82[:, :], 2)

            xoff = 0
            yoff = 0
            n_t = 0
            for tf, tg in plan:
                # tiles of different sizes share slots via a common tag,
                # sized to the largest tile
                xt = xp.tile([P, tf + PAD], f32, tag="xt")
                # first use of each pool slot: initialize the one pad element
                # the shifted stream reads (keeps CoreSim's checker happy; the
                # affected rows are recomputed on the host regardless)
                if n_t < BUFS:
                    nc.vector.memset(xt[:, PAD - 1:PAD], 0.0)
                n_t += 1

                xin = x[xoff:xoff + P * tf].rearrange("(p f) -> p f", p=P)
                nc.gpsimd.dma_start(out=xt[:, PAD:], in_=xin)
                x3 = xt[:, PAD:].rearrange("p (j c) -> p j c", c=C)
                xs3 = xt[:, PAD - 1:tf + PAD - 1].rearrange(
                    "p (j c) -> p j c", c=C)

                runc = rp.tile([P, tg], u16, tag="rc")
                rc3 = runc[:].unsqueeze(2).broadcast_to([P, tg, C])
                nc.vector._custom_dve(
                    argmax_op, out=rc3, in0=x3, in1=xs3, s0=0.0, s1=0.0)

                # on-device decode: idx = runc - (8j + 2), shipped as uint8
                o = op_.tile([P, tg], u8, tag="o")
                nc.vector.tensor_tensor(
                    out=o[:], in0=runc[:], in1=i82[:, :tg], op=Alu.subtract)
                yout = y[yoff:yoff + P * tg].rearrange("(p j) -> p j", p=P)
                nc.gpsimd.dma_start(out=yout, in_=o[:])
                xoff += P * tf
                yoff += P * tg
    nc.finalize()
    return nc


def _get_nc(n_tiles=N_TILES):
    if n_tiles not in _NC_CACHE:
        _NC_CACHE[n_tiles] = _build_nc(n_tiles)
    return _NC_CACHE[n_tiles]


def _decode_core(idx_core, x_core, n_tiles=N_TILES):
    """Fix up one core's device argmaxes (uint8).

    The first row of every (tile, partition) block is recomputed from
    x_core (its shifted stream read one pad element on device).
    """
    out = idx_core.astype(np.int64)
    off = 0
    for tf, tg in _tile_plan(n_tiles):
        first = off + np.arange(P) * tg  # row ids with j == 0
        out[first] = np.argmax(x_core[first], axis=1)
        off += P * tg
    return out


def kernel(inputs, label_table):
    x = np.ascontiguousarray(np.asarray(inputs, dtype=np.float32))
    lt = np.asarray(label_table)
    assert x.shape == (ROWS, C), x.shape

    from concourse.bass_utils import run_bass_kernel_spmd

    nc = _get_nc()
    in_maps = [
        {"x": x[i * ROWS_PER_CORE:(i + 1) * ROWS_PER_CORE].reshape(-1)}
        for i in range(N_CORES)
    ]
    res = run_bass_kernel_spmd(nc, in_maps, core_ids=list(range(N_CORES)))
    idx = np.concatenate([
        _decode_core(
            np.asarray(res.results[i]["y"]).reshape(-1),
            x[i * ROWS_PER_CORE:(i + 1) * ROWS_PER_CORE],
        )
        for i in range(N_CORES)
    ])
    return np.take(lt, idx).astype(lt.dtype)
